# revision 19
# baseline (speedup 1.0000x reference)
"""Trainium2 Bass kernel for nn_CAM_41377714929724 (CAM cross-attention module).

  a1  = f1 @ W                      [B,S,D]
  cc  = a1 @ f2^T                   [B,S,S]
  aatt = softmax(cc, axis=s)        (over rows)
  vatt = softmax(cc, axis=t).T      (over cols, transposed)
  out1 = (f1 @ aatt).swap(1,2)      [B,S,S]
  out2 = (f2 @ vatt).swap(1,2)      [B,S,S]

Sharding: pure data parallelism, 2 batches per core on 8 cores; W replicated.

Per core/batch dataflow (all matmuls fp32r = full PE rate, fp32 PSUM accum):
  a1T[e,s] = sum_d W[d,e] f1T[d,s]          (lhsT=W,    rhs=f1T)
  cc [s,t] = sum_e a1T[e,s] f2T[e,t]        (lhsT=a1T,  rhs=f2T)
  vmax[s]  = max_t cc   (DVE free-dim reduce -> [128,8] stat tile
                         -> PE-transpose -> -vmax row, no DRAM bounce)
  ccT[t,s] = sum_e f2T[e,t] a1T[e,s] - vmax[s]
             (K=1 ones x (-vmax row) matmul appended to the accumulation;
              PSUM drain IS the exp -> e2T[t,s] in one ACT op)
  amax[t]  = max_s cc   (DVE max-combine of 8 tiles + 1 gpsimd partition allreduce)
  e1 [u,t] = exp(cc - amax[t])  in place    (DVE sub + ACT exp)
  asum[x]  = sum_u e1[u,x]  (8x N=1 matmul vs ones column -> [128,1] PSUM,
                             per-partition x directly; 1/asum via DVE recip)
  vsum[s]  = sum_u e2T[u,s] (same)
  out1[x,s] = (sum_u e1[u,x] f1T[u,s]) * (1/asum[x])  (scale fused in PSUM drain)
  out2[s,t] = (sum_u e2T[u,s] f2T[u,t]) * (1/vsum[s])

Column-halved stats keep the PE dense: ret matmuls of one half start while the
other half's stats are in flight.
"""

import numpy as np
from contextlib import ExitStack

import concourse.bass as bass
import concourse.tile as tile
from concourse import bacc, mybir, bass_isa
from concourse.bass_utils import run_bass_kernel_spmd

f32 = mybir.dt.float32
f32r = mybir.dt.float32r

P = 128
N = 1024
NT = N // P          # 8 tiles per matrix dim
NB = 2               # batches per core
NCORES = 8
HALF = 512           # matmul moving free dim / psum bank
Exp = mybir.ActivationFunctionType.Exp
Copy = mybir.ActivationFunctionType.Copy


def _build():
    nc = bacc.Bacc("TRN2", target_bir_lowering=False, debug=False, num_devices=NCORES)

    f1t_d = nc.dram_tensor("f1t", [NB, N, N], f32r, kind="ExternalInput").ap()
    f2t_d = nc.dram_tensor("f2t", [NB, N, N], f32r, kind="ExternalInput").ap()
    w_d = nc.dram_tensor("w", [N, N], f32r, kind="ExternalInput").ap()
    o1_d = nc.dram_tensor("o1", [NB, N, N], f32, kind="ExternalOutput").ap()
    o2_d = nc.dram_tensor("o2", [NB, N, N], f32, kind="ExternalOutput").ap()

    with tile.TileContext(nc) as tc, ExitStack() as ctx:
        wp = ctx.enter_context(tc.tile_pool(name="wp", bufs=1))
        f1p = ctx.enter_context(tc.tile_pool(name="f1p", bufs=1))
        f2p = ctx.enter_context(tc.tile_pool(name="f2p", bufs=1))
        a1p = ctx.enter_context(tc.tile_pool(name="a1p", bufs=1))
        ccp = ctx.enter_context(tc.tile_pool(name="ccp", bufs=1))
        cctp = ctx.enter_context(tc.tile_pool(name="cctp", bufs=1))
        statp = ctx.enter_context(tc.tile_pool(name="statp", bufs=1))
        smallp = ctx.enter_context(tc.tile_pool(name="smallp", bufs=1))
        oretp = ctx.enter_context(tc.tile_pool(name="oretp", bufs=2))
        psp = ctx.enter_context(tc.tile_pool(name="psp", bufs=6, space="PSUM"))
        rowpsp = ctx.enter_context(tc.tile_pool(name="rowpsp", bufs=2, space="PSUM"))
        dscrp = ctx.enter_context(tc.tile_pool(name="dscrp", bufs=2, space="DRAM"))

        # constants: fp32r ones (memset can't write f32r), fp32 identity
        ones_f32r_ = smallp.tile([1, P], f32, name="ones_f32r_", tag="ones_f32r_")
        nc.vector.memset(ones_f32r_[:], 1.0)
        ones_k1 = smallp.tile([1, P], f32r, name="ones_k1", tag="ones_k1")
        nc.scalar.copy(ones_k1[:], ones_f32r_[:])
        ones_f32c_ = smallp.tile([P, 1], f32, name="ones_f32c_", tag="ones_f32c_")
        nc.vector.memset(ones_f32c_[:], 1.0)
        ones_col = smallp.tile([P, 1], f32r, name="ones_col", tag="ones_col")
        nc.scalar.copy(ones_col[:], ones_f32c_[:])

        # W is shared by both batches: load once
        ws = []
        f1s_by_b = {}
        for k in range(NT):
            wk = wp.tile([P, N], f32r, name=f"w{k}", tag=f"w{k}")
            nc.sync.dma_start(wk[:], w_d[k * P:(k + 1) * P, :])
            ws.append(wk)
            f1k = f1p.tile([P, N], f32r, name=f"f1_0_{k}", tag=f"f1{k}")
            nc.sync.dma_start(f1k[:], f1t_d[0, k * P:(k + 1) * P, :])
            f1s_by_b.setdefault(0, []).append(f1k)

        for b in range(NB):
            # ---- loads -------------------------------------------------
            if b == 0:
                f1s = f1s_by_b[0]
            else:
                f1s = []
                for k in range(NT):
                    f1k = f1p.tile([P, N], f32r, name=f"f1_{b}_{k}", tag=f"f1{k}")
                    nc.sync.dma_start(f1k[:], f1t_d[b, k * P:(k + 1) * P, :])
                    f1s.append(f1k)
            f2s = []
            for k in range(NT):
                f2k = f2p.tile([P, N], f32r, name=f"f2_{b}_{k}", tag=f"f2{k}")
                nc.sync.dma_start(f2k[:], f2t_d[b, k * P:(k + 1) * P, :])
                f2s.append(f2k)

            def mmgroup(lhs_tiles, rhs_tiles, m, n, drain, tagpfx, extra=None):
                ps = psp.tile([P, HALF], f32, name=f"ps_{tagpfx}", tag="ps")
                for k in range(NT):
                    nc.tensor.matmul(
                        ps[:],
                        lhs_tiles[k][:, m * P:(m + 1) * P],
                        rhs_tiles[k][:, n * HALF:(n + 1) * HALF],
                        start=(k == 0),
                        stop=(k == NT - 1 and extra is None),
                    )
                if extra is not None:
                    extra(ps)
                drain(m, n, ps)

            # ---- a1T[e,s] ----------------------------------------------
            a1s = [a1p.tile([P, N], f32r, name=f"a1_{b}_{m}", tag=f"a1{m}")
                   for m in range(NT)]
            for m in range(NT):
                for n in range(2):
                    mmgroup(ws, f1s, m, n,
                            lambda m_, n_, ps: nc.scalar.copy(
                                a1s[m_][:, n_ * HALF:(n_ + 1) * HALF], ps[:]),
                            "a1")

            # ---- cc[s,t] + vmax stat tile -------------------------------
            ccs = [ccp.tile([P, N], f32r, name=f"cc_{b}_{m}", tag=f"cc{m}")
                   for m in range(NT)]
            nvmax_pp = [smallp.tile([P, 1], f32r, name=f"nvmax_{b}_{m}",
                                    tag=f"vmaxpp{m}") for m in range(NT)]
            scr_v = dscrp.tile([1, N], f32r, name=f"scr_v{b}", tag="scr_v")
            nvrow = statp.tile([1, N], f32r, name=f"nvrow{b}", tag="nvrow")

            def cc_drain(m, n, ps):
                nc.vector.tensor_copy(ccs[m][:, n * HALF:(n + 1) * HALF], ps[:])

            for m in range(NT):
                for n in range(2):
                    mmgroup(a1s, f2s, m, n, cc_drain, "cc")
                nc.vector.tensor_reduce(
                    out=nvmax_pp[m][:], in_=ccs[m][:].bitcast(f32),
                    axis=mybir.AxisListType.X, op=mybir.AluOpType.max, negate=True)
                nc.sync.dma_start(
                    scr_v[0:1, m * P:(m + 1) * P].rearrange("one (p x) -> (one p) x", p=P),
                    nvmax_pp[m][:])
            for h in range(2):
                nc.sync.dma_start(nvrow[0:1, h * HALF:(h + 1) * HALF],
                                  scr_v[0:1, h * HALF:(h + 1) * HALF])

            # ---- ccT[t,s] - vmax[s], exp-drained -> e2T ------------------
            ccts = [cctp.tile([P, N], f32r, name=f"cct_{b}_{m}", tag=f"cct{m}")
                    for m in range(NT)]

            def cct_drain(m, n, ps):
                nc.scalar.activation(ccts[m][:, n * HALF:(n + 1) * HALF], ps[:], Exp)

            for m in range(NT):
                for n in range(2):
                    mmgroup(f2s, a1s, m, n, cct_drain, "cct",
                            extra=lambda ps, n_=n: nc.tensor.matmul(
                                ps[:], ones_k1[:],
                                nvrow[0:1, n_ * HALF:(n_ + 1) * HALF],
                                start=False, stop=True))

            # ---- amax + e1 = exp(cc - amax), per column half -------------
            for h in range(2):
                sl = slice(h * HALF, (h + 1) * HALF)
                amaxt = statp.tile([P, HALF], f32, name=f"amaxt{b}{h}", tag="amaxt")
                nc.vector.tensor_copy(amaxt[:], ccs[0][:, sl].bitcast(f32))
                for m in range(1, NT):
                    nc.vector.tensor_tensor(
                        out=amaxt[:], in0=amaxt[:], in1=ccs[m][:, sl].bitcast(f32),
                        op=mybir.AluOpType.max)
                amaxB = statp.tile([P, HALF], f32, name=f"amaxB{b}{h}", tag="amaxB")
                nc.gpsimd.partition_all_reduce(
                    amaxB[:], amaxt[:], channels=P, reduce_op=bass_isa.ReduceOp.max)
                for m in range(NT):
                    nc.vector.tensor_tensor(
                        out=ccs[m][:, sl], in0=ccs[m][:, sl].bitcast(f32),
                        in1=amaxB[:], op=mybir.AluOpType.subtract)
                    nc.scalar.activation(ccs[m][:, sl], ccs[m][:, sl].bitcast(f32), Exp)

            # ---- per-half: column sums via N=1 matmuls, then ret ---------
            rsa = smallp.tile([P, NT], f32, name=f"rsa{b}", tag="rsa")
            rsv = smallp.tile([P, NT], f32, name=f"rsv{b}", tag="rsv")

            def colsum_mm(tiles, m, rs):
                # fp32 matmul (fp32r rejects N=1); K=1024, N=1 -> negligible PE time
                sps = rowpsp.tile([P, 1], f32, name="sps", tag="rowps")
                for k in range(NT):
                    nc.tensor.matmul(sps[:],
                                     tiles[k][:, m * P:(m + 1) * P].bitcast(f32),
                                     ones_f32c_[:], start=(k == 0), stop=(k == NT - 1))
                nc.scalar.copy(rs[:, m:m + 1], sps[:])

            def ret_drain(out_d, rs):
                def d(m, n, ps):
                    ot = oretp.tile([P, HALF], f32, name="oret", tag="oret")
                    nc.scalar.activation(ot[:], ps[:], Copy,
                                         bias=0.0, scale=rs[:, m:m + 1])
                    nc.sync.dma_start(
                        out_d[b, m * P:(m + 1) * P, n * HALF:(n + 1) * HALF], ot[:])
                return d

            for h in range(2):
                for m in range(4 * h, 4 * h + 4):
                    colsum_mm(ccs, m, rsa)
                nc.vector.reciprocal(rsa[:, 4 * h:4 * h + 4], rsa[:, 4 * h:4 * h + 4])
                for m in range(4 * h, 4 * h + 4):
                    for n in range(2):
                        mmgroup(ccs, f1s, m, n, ret_drain(o1_d, rsa), "r1")

            for h in range(2):
                for m in range(4 * h, 4 * h + 4):
                    colsum_mm(ccts, m, rsv)
                nc.vector.reciprocal(rsv[:, 4 * h:4 * h + 4], rsv[:, 4 * h:4 * h + 4])
                for m in range(4 * h, 4 * h + 4):
                    for n in range(2):
                        mmgroup(ccts, f2s, m, n, ret_drain(o2_d, rsv), "r2")

    nc.compile()
    return nc


_NC = None
TRACE = False
LAST = None


def _get_nc():
    global _NC
    if _NC is None:
        _NC = _build()
    return _NC


def kernel(f1_norm, f2_norm, corr_weights):
    f1_norm = np.ascontiguousarray(f1_norm, dtype=np.float32)
    f2_norm = np.ascontiguousarray(f2_norm, dtype=np.float32)
    w = np.ascontiguousarray(corr_weights, dtype=np.float32)
    B = f1_norm.shape[0]
    assert B == NB * NCORES

    # host-side feature-major transposes: f1t[b] = f1[b].T
    f1t = np.ascontiguousarray(np.swapaxes(f1_norm, 1, 2))
    f2t = np.ascontiguousarray(np.swapaxes(f2_norm, 1, 2))

    nc = _get_nc()
    in_maps = [
        {"f1t": f1t[c * NB:(c + 1) * NB], "f2t": f2t[c * NB:(c + 1) * NB], "w": w}
        for c in range(NCORES)
    ]
    res = run_bass_kernel_spmd(nc, in_maps, core_ids=list(range(NCORES)), trace=TRACE)
    global LAST
    LAST = res
    out1 = np.concatenate([res.results[c]["o1"] for c in range(NCORES)], axis=0)
    out2 = np.concatenate([res.results[c]["o2"] for c in range(NCORES)], axis=0)
    return out1, out2


# revision 21
# speedup vs baseline: 1.2149x; 1.2149x over previous
"""Trainium2 Bass kernel for nn_CAM_41377714929724 (CAM cross-attention module).

  a1  = f1 @ W                      [B,S,D]
  cc  = a1 @ f2^T                   [B,S,S]
  aatt = softmax(cc, axis=s)        (over rows)
  vatt = softmax(cc, axis=t).T      (over cols, transposed)
  out1 = (f1 @ aatt).swap(1,2)      [B,S,S]
  out2 = (f2 @ vatt).swap(1,2)      [B,S,S]

Sharding: pure data parallelism, 2 batches per core on 8 cores; W replicated.

Per core/batch dataflow (all matmuls fp32r = full PE rate, fp32 PSUM accum):
  a1T[e,s] = sum_d W[d,e] f1T[d,s]          (lhsT=W,    rhs=f1T)
  cc [s,t] = sum_e a1T[e,s] f2T[e,t]        (lhsT=a1T,  rhs=f2T)
  vmax[s]  = max_t cc   (DVE free-dim reduce -> [128,8] stat tile
                         -> PE-transpose -> -vmax row, no DRAM bounce)
  ccT[t,s] = sum_e f2T[e,t] a1T[e,s] - vmax[s]
             (K=1 ones x (-vmax row) matmul appended to the accumulation;
              PSUM drain IS the exp -> e2T[t,s] in one ACT op)
  amax[t]  = max_s cc   (DVE max-combine of 8 tiles + 1 gpsimd partition allreduce)
  e1 [u,t] = exp(cc - amax[t])  in place    (DVE sub + ACT exp)
  asum[x]  = sum_u e1[u,x]  (8x N=1 matmul vs ones column -> [128,1] PSUM,
                             per-partition x directly; 1/asum via DVE recip)
  vsum[s]  = sum_u e2T[u,s] (same)
  out1[x,s] = (sum_u e1[u,x] f1T[u,s]) * (1/asum[x])  (scale fused in PSUM drain)
  out2[s,t] = (sum_u e2T[u,s] f2T[u,t]) * (1/vsum[s])

Column-halved stats keep the PE dense: ret matmuls of one half start while the
other half's stats are in flight.
"""

import numpy as np
from contextlib import ExitStack

import concourse.bass as bass
import concourse.tile as tile
from concourse import bacc, mybir, bass_isa
from concourse.bass_utils import run_bass_kernel_spmd

f32 = mybir.dt.float32
f32r = mybir.dt.float32r

P = 128
N = 1024
NT = N // P          # 8 tiles per matrix dim
NB = 2               # batches per core
NCORES = 8
HALF = 512           # matmul moving free dim / psum bank
Exp = mybir.ActivationFunctionType.Exp
Copy = mybir.ActivationFunctionType.Copy


def _build():
    nc = bacc.Bacc("TRN2", target_bir_lowering=False, debug=False, num_devices=NCORES)

    f1t_d = nc.dram_tensor("f1t", [NB, N, N], f32r, kind="ExternalInput").ap()
    f2t_d = nc.dram_tensor("f2t", [NB, N, N], f32r, kind="ExternalInput").ap()
    w_d = nc.dram_tensor("w", [N, N], f32r, kind="ExternalInput").ap()
    o1_d = nc.dram_tensor("o1", [NB, N, N], f32, kind="ExternalOutput").ap()
    o2_d = nc.dram_tensor("o2", [NB, N, N], f32, kind="ExternalOutput").ap()

    with tile.TileContext(nc) as tc, ExitStack() as ctx:
        wp = ctx.enter_context(tc.tile_pool(name="wp", bufs=1))
        f1p = ctx.enter_context(tc.tile_pool(name="f1p", bufs=1))
        f2p = ctx.enter_context(tc.tile_pool(name="f2p", bufs=1))
        a1p = ctx.enter_context(tc.tile_pool(name="a1p", bufs=1))
        ccp = ctx.enter_context(tc.tile_pool(name="ccp", bufs=1))
        cctp = ctx.enter_context(tc.tile_pool(name="cctp", bufs=1))
        statp = ctx.enter_context(tc.tile_pool(name="statp", bufs=1))
        smallp = ctx.enter_context(tc.tile_pool(name="smallp", bufs=1))
        oretp = ctx.enter_context(tc.tile_pool(name="oretp", bufs=2))
        psp = ctx.enter_context(tc.tile_pool(name="psp", bufs=7, space="PSUM"))
        rowpsp = ctx.enter_context(tc.tile_pool(name="rowpsp", bufs=1, space="PSUM"))
        dscrp = ctx.enter_context(tc.tile_pool(name="dscrp", bufs=2, space="DRAM"))

        # constants: fp32r ones (memset can't write f32r), fp32 identity
        ones_f32r_ = smallp.tile([1, P], f32, name="ones_f32r_", tag="ones_f32r_")
        nc.vector.memset(ones_f32r_[:], 1.0)
        ones_k1 = smallp.tile([1, P], f32r, name="ones_k1", tag="ones_k1")
        nc.scalar.copy(ones_k1[:], ones_f32r_[:])
        ones_f32c_ = smallp.tile([P, 1], f32, name="ones_f32c_", tag="ones_f32c_")
        nc.vector.memset(ones_f32c_[:], 1.0)
        ones_col = smallp.tile([P, 1], f32r, name="ones_col", tag="ones_col")
        nc.scalar.copy(ones_col[:], ones_f32c_[:])

        # W is shared by both batches: load once
        ws = []
        f1s_by_b = {}
        for k in range(NT):
            wk = wp.tile([P, N], f32r, name=f"w{k}", tag=f"w{k}")
            nc.sync.dma_start(wk[:], w_d[k * P:(k + 1) * P, :])
            ws.append(wk)
            f1k = f1p.tile([P, N], f32r, name=f"f1_0_{k}", tag=f"f1{k}")
            nc.sync.dma_start(f1k[:], f1t_d[0, k * P:(k + 1) * P, :])
            f1s_by_b.setdefault(0, []).append(f1k)

        for b in range(NB):
            # ---- loads -------------------------------------------------
            if b == 0:
                f1s = f1s_by_b[0]
            else:
                f1s = []
                for k in range(NT):
                    f1k = f1p.tile([P, N], f32r, name=f"f1_{b}_{k}", tag=f"f1{k}")
                    nc.sync.dma_start(f1k[:], f1t_d[b, k * P:(k + 1) * P, :])
                    f1s.append(f1k)
            f2s = []
            for k in range(NT):
                f2k = f2p.tile([P, N], f32r, name=f"f2_{b}_{k}", tag=f"f2{k}")
                nc.sync.dma_start(f2k[:], f2t_d[b, k * P:(k + 1) * P, :])
                f2s.append(f2k)

            def mmgroup(lhs_tiles, rhs_tiles, m, n, drain, tagpfx, extra=None):
                ps = psp.tile([P, HALF], f32, name=f"ps_{tagpfx}", tag="ps")
                for k in range(NT):
                    nc.tensor.matmul(
                        ps[:],
                        lhs_tiles[k][:, m * P:(m + 1) * P],
                        rhs_tiles[k][:, n * HALF:(n + 1) * HALF],
                        start=(k == 0),
                        stop=(k == NT - 1 and extra is None),
                    )
                if extra is not None:
                    extra(ps)
                drain(m, n, ps)

            # ---- a1T[e,s] ----------------------------------------------
            a1s = [a1p.tile([P, N], f32r, name=f"a1_{b}_{m}", tag=f"a1{m}")
                   for m in range(NT)]
            for m in range(NT):
                for n in range(2):
                    mmgroup(ws, f1s, m, n,
                            lambda m_, n_, ps: nc.scalar.copy(
                                a1s[m_][:, n_ * HALF:(n_ + 1) * HALF], ps[:]),
                            "a1")

            # ---- cc[s,t] + vmax stat tile -------------------------------
            ccs = [ccp.tile([P, N], f32r, name=f"cc_{b}_{m}", tag=f"cc{m}")
                   for m in range(NT)]
            nvmax_pp = [smallp.tile([P, 1], f32r, name=f"nvmax_{b}_{m}",
                                    tag=f"vmaxpp{m}") for m in range(NT)]
            scr_v = dscrp.tile([1, N], f32r, name=f"scr_v{b}", tag="scr_v")
            nvrow = statp.tile([1, N], f32r, name=f"nvrow{b}", tag="nvrow")

            def cc_drain(m, n, ps):
                nc.vector.tensor_copy(ccs[m][:, n * HALF:(n + 1) * HALF], ps[:])

            for m in range(NT):
                for n in range(2):
                    mmgroup(a1s, f2s, m, n, cc_drain, "cc")
                nc.vector.tensor_reduce(
                    out=nvmax_pp[m][:], in_=ccs[m][:].bitcast(f32),
                    axis=mybir.AxisListType.X, op=mybir.AluOpType.max, negate=True)
                nc.sync.dma_start(
                    scr_v[0:1, m * P:(m + 1) * P].rearrange("one (p x) -> (one p) x", p=P),
                    nvmax_pp[m][:])
            for h in range(2):
                nc.sync.dma_start(nvrow[0:1, h * HALF:(h + 1) * HALF],
                                  scr_v[0:1, h * HALF:(h + 1) * HALF])

            # ---- ccT[t,s] - vmax[s], exp-drained -> e2T ------------------
            ccts = [cctp.tile([P, N], f32r, name=f"cct_{b}_{m}", tag=f"cct{m}")
                    for m in range(NT)]

            def cct_drain(m, n, ps):
                nc.scalar.activation(ccts[m][:, n * HALF:(n + 1) * HALF], ps[:], Exp)

            for m in range(NT):
                for n in range(2):
                    mmgroup(f2s, a1s, m, n, cct_drain, "cct",
                            extra=lambda ps, n_=n: nc.tensor.matmul(
                                ps[:], ones_k1[:],
                                nvrow[0:1, n_ * HALF:(n_ + 1) * HALF],
                                start=False, stop=True))

            # ---- amax + e1 = exp(cc - amax), per column half -------------
            for h in range(2):
                sl = slice(h * HALF, (h + 1) * HALF)
                amaxt = statp.tile([P, HALF], f32, name=f"amaxt{b}{h}", tag="amaxt")
                nc.vector.tensor_copy(amaxt[:], ccs[0][:, sl].bitcast(f32))
                for m in range(1, NT):
                    nc.vector.tensor_tensor(
                        out=amaxt[:], in0=amaxt[:], in1=ccs[m][:, sl].bitcast(f32),
                        op=mybir.AluOpType.max)
                amaxB = statp.tile([P, HALF], f32, name=f"amaxB{b}{h}", tag="amaxB")
                nc.gpsimd.partition_all_reduce(
                    amaxB[:], amaxt[:], channels=P, reduce_op=bass_isa.ReduceOp.max)
                for m in range(NT):
                    nc.vector.tensor_tensor(
                        out=ccs[m][:, sl], in0=ccs[m][:, sl].bitcast(f32),
                        in1=amaxB[:], op=mybir.AluOpType.subtract)
                    nc.scalar.activation(ccs[m][:, sl], ccs[m][:, sl].bitcast(f32), Exp)

            # ---- column sums via f32r ones-row matmuls + DRAM bounce -----
            rsa = smallp.tile([P, NT], f32, name=f"rsa{b}", tag="rsa")
            rsv = smallp.tile([P, NT], f32, name=f"rsv{b}", tag="rsv")
            scr_s = dscrp.tile([1, 4 * N], f32, name=f"scr_s{b}", tag="scr_s")

            def colsum_mm(tiles, h, col):
                sps = rowpsp.tile([1, HALF], f32, name="sps", tag="rowps")
                for k in range(NT):
                    nc.tensor.matmul(
                        sps[:], ones_col[:], tiles[k][:, h * HALF:(h + 1) * HALF],
                        start=(k == 0), stop=(k == NT - 1))
                # hop through SBUF (DMA cannot read PSUM); reuse a dead a1 slot
                srow = a1p.tile([1, HALF], f32, name="sumrow", tag="a10")
                nc.scalar.copy(srow[:], sps[:])
                nc.sync.dma_start(scr_s[0:1, col * HALF:(col + 1) * HALF], srow[:])

            def ret_drain(out_d, rs):
                def d(m, n, ps):
                    ot = oretp.tile([P, HALF], f32, name="oret", tag="oret")
                    nc.scalar.activation(ot[:], ps[:], Copy,
                                         bias=0.0, scale=rs[:, m:m + 1])
                    nc.sync.dma_start(
                        out_d[b, m * P:(m + 1) * P, n * HALF:(n + 1) * HALF], ot[:])
                return d

            for h in range(2):
                colsum_mm(ccs, h, h)          # asum halves at scr_s cols 0,1
            for h in range(2):
                rd = scr_s[0:1, h * HALF:(h + 1) * HALF].rearrange(
                    "one (m p) -> (one p) m", p=P)
                nc.sync.dma_start(rsa[:, 4 * h:4 * h + 4], rd)
            nc.vector.reciprocal(rsa[:], rsa[:])

            for m in range(NT):
                for n in range(2):
                    mmgroup(ccs, f1s, m, n, ret_drain(o1_d, rsa), "r1")

            for h in range(2):
                colsum_mm(ccts, h, 2 + h)     # vsum halves at scr_s cols 2,3
            for h in range(2):
                rd = scr_s[0:1, (2 + h) * HALF:(3 + h) * HALF].rearrange(
                    "one (m p) -> (one p) m", p=P)
                nc.sync.dma_start(rsv[:, 4 * h:4 * h + 4], rd)
            nc.vector.reciprocal(rsv[:], rsv[:])

            for m in range(NT):
                for n in range(2):
                    mmgroup(ccts, f2s, m, n, ret_drain(o2_d, rsv), "r2")

    nc.compile()
    return nc


_NC = None
TRACE = False
LAST = None


def _get_nc():
    global _NC
    if _NC is None:
        _NC = _build()
    return _NC


def kernel(f1_norm, f2_norm, corr_weights):
    f1_norm = np.ascontiguousarray(f1_norm, dtype=np.float32)
    f2_norm = np.ascontiguousarray(f2_norm, dtype=np.float32)
    w = np.ascontiguousarray(corr_weights, dtype=np.float32)
    B = f1_norm.shape[0]
    assert B == NB * NCORES

    # host-side feature-major transposes: f1t[b] = f1[b].T
    f1t = np.ascontiguousarray(np.swapaxes(f1_norm, 1, 2))
    f2t = np.ascontiguousarray(np.swapaxes(f2_norm, 1, 2))

    nc = _get_nc()
    in_maps = [
        {"f1t": f1t[c * NB:(c + 1) * NB], "f2t": f2t[c * NB:(c + 1) * NB], "w": w}
        for c in range(NCORES)
    ]
    res = run_bass_kernel_spmd(nc, in_maps, core_ids=list(range(NCORES)), trace=TRACE)
    global LAST
    LAST = res
    out1 = np.concatenate([res.results[c]["o1"] for c in range(NCORES)], axis=0)
    out2 = np.concatenate([res.results[c]["o2"] for c in range(NCORES)], axis=0)
    return out1, out2


# revision 22
# speedup vs baseline: 1.2356x; 1.0171x over previous
"""Trainium2 Bass kernel for nn_CAM_41377714929724 (CAM cross-attention module).

  a1  = f1 @ W                      [B,S,D]
  cc  = a1 @ f2^T                   [B,S,S]
  aatt = softmax(cc, axis=s)        (over rows)
  vatt = softmax(cc, axis=t).T      (over cols, transposed)
  out1 = (f1 @ aatt).swap(1,2)      [B,S,S]
  out2 = (f2 @ vatt).swap(1,2)      [B,S,S]

Sharding: pure data parallelism, 2 batches per core on 8 cores; W replicated.

Per core/batch dataflow (all matmuls fp32r = full PE rate, fp32 PSUM accum):
  a1T[e,s] = sum_d W[d,e] f1T[d,s]          (lhsT=W,    rhs=f1T)
  cc [s,t] = sum_e a1T[e,s] f2T[e,t]        (lhsT=a1T,  rhs=f2T)
  vmax[s]  = max_t cc   (DVE free-dim reduce -> [128,8] stat tile
                         -> PE-transpose -> -vmax row, no DRAM bounce)
  ccT[t,s] = sum_e f2T[e,t] a1T[e,s] - vmax[s]
             (K=1 ones x (-vmax row) matmul appended to the accumulation;
              PSUM drain IS the exp -> e2T[t,s] in one ACT op)
  amax[t]  = max_s cc   (DVE max-combine of 8 tiles + 1 gpsimd partition allreduce)
  e1 [u,t] = exp(cc - amax[t])  in place    (DVE sub + ACT exp)
  asum[x]  = sum_u e1[u,x]  (8x N=1 matmul vs ones column -> [128,1] PSUM,
                             per-partition x directly; 1/asum via DVE recip)
  vsum[s]  = sum_u e2T[u,s] (same)
  out1[x,s] = (sum_u e1[u,x] f1T[u,s]) * (1/asum[x])  (scale fused in PSUM drain)
  out2[s,t] = (sum_u e2T[u,s] f2T[u,t]) * (1/vsum[s])

Column-halved stats keep the PE dense: ret matmuls of one half start while the
other half's stats are in flight.
"""

import numpy as np
from contextlib import ExitStack

import concourse.bass as bass
import concourse.tile as tile
from concourse import bacc, mybir, bass_isa
from concourse.bass_utils import run_bass_kernel_spmd

f32 = mybir.dt.float32
f32r = mybir.dt.float32r

P = 128
N = 1024
NT = N // P          # 8 tiles per matrix dim
NB = 2               # batches per core
NCORES = 8
HALF = 512           # matmul moving free dim / psum bank
Exp = mybir.ActivationFunctionType.Exp
Copy = mybir.ActivationFunctionType.Copy


def _build():
    nc = bacc.Bacc("TRN2", target_bir_lowering=False, debug=False, num_devices=NCORES)

    f1t_d = nc.dram_tensor("f1t", [NB, N, N], f32r, kind="ExternalInput").ap()
    f2t_d = nc.dram_tensor("f2t", [NB, N, N], f32r, kind="ExternalInput").ap()
    w_d = nc.dram_tensor("w", [N, N], f32r, kind="ExternalInput").ap()
    o1_d = nc.dram_tensor("o1", [NB, N, N], f32, kind="ExternalOutput").ap()
    o2_d = nc.dram_tensor("o2", [NB, N, N], f32, kind="ExternalOutput").ap()

    with tile.TileContext(nc) as tc, ExitStack() as ctx:
        wp = ctx.enter_context(tc.tile_pool(name="wp", bufs=1))
        f1p = ctx.enter_context(tc.tile_pool(name="f1p", bufs=1))
        f2p = ctx.enter_context(tc.tile_pool(name="f2p", bufs=1))
        a1p = ctx.enter_context(tc.tile_pool(name="a1p", bufs=1))
        ccp = ctx.enter_context(tc.tile_pool(name="ccp", bufs=1))
        cctp = ctx.enter_context(tc.tile_pool(name="cctp", bufs=1))
        statp = ctx.enter_context(tc.tile_pool(name="statp", bufs=1))
        smallp = ctx.enter_context(tc.tile_pool(name="smallp", bufs=1))
        oretp = ctx.enter_context(tc.tile_pool(name="oretp", bufs=2))
        psp = ctx.enter_context(tc.tile_pool(name="psp", bufs=7, space="PSUM"))
        rowpsp = ctx.enter_context(tc.tile_pool(name="rowpsp", bufs=1, space="PSUM"))
        dscrp = ctx.enter_context(tc.tile_pool(name="dscrp", bufs=2, space="DRAM"))

        # constants: fp32r ones (memset can't write f32r), fp32 identity
        ones_f32r_ = smallp.tile([1, P], f32, name="ones_f32r_", tag="ones_f32r_")
        nc.vector.memset(ones_f32r_[:], 1.0)
        ones_k1 = smallp.tile([1, P], f32r, name="ones_k1", tag="ones_k1")
        nc.scalar.copy(ones_k1[:], ones_f32r_[:])
        ones_f32c_ = smallp.tile([P, 1], f32, name="ones_f32c_", tag="ones_f32c_")
        nc.vector.memset(ones_f32c_[:], 1.0)
        ones_col = smallp.tile([P, 1], f32r, name="ones_col", tag="ones_col")
        nc.scalar.copy(ones_col[:], ones_f32c_[:])

        # W is shared by both batches: load once
        ws = []
        f1s_by_b = {}
        for k in range(NT):
            wk = wp.tile([P, N], f32r, name=f"w{k}", tag=f"w{k}")
            nc.sync.dma_start(wk[:], w_d[k * P:(k + 1) * P, :])
            ws.append(wk)
            f1k = f1p.tile([P, N], f32r, name=f"f1_0_{k}", tag=f"f1{k}")
            nc.sync.dma_start(f1k[:], f1t_d[0, k * P:(k + 1) * P, :])
            f1s_by_b.setdefault(0, []).append(f1k)

        for b in range(NB):
            # ---- loads -------------------------------------------------
            if b == 0:
                f1s = f1s_by_b[0]
            else:
                f1s = []
                for k in range(NT):
                    f1k = f1p.tile([P, N], f32r, name=f"f1_{b}_{k}", tag=f"f1{k}")
                    nc.sync.dma_start(f1k[:], f1t_d[b, k * P:(k + 1) * P, :])
                    f1s.append(f1k)
            f2s = []
            for k in range(NT):
                f2k = f2p.tile([P, N], f32r, name=f"f2_{b}_{k}", tag=f"f2{k}")
                nc.sync.dma_start(f2k[:], f2t_d[b, k * P:(k + 1) * P, :])
                f2s.append(f2k)

            def mmgroup(lhs_tiles, rhs_tiles, m, n, drain, tagpfx, extra=None):
                ps = psp.tile([P, HALF], f32, name=f"ps_{tagpfx}", tag="ps")
                for k in range(NT):
                    nc.tensor.matmul(
                        ps[:],
                        lhs_tiles[k][:, m * P:(m + 1) * P],
                        rhs_tiles[k][:, n * HALF:(n + 1) * HALF],
                        start=(k == 0),
                        stop=(k == NT - 1 and extra is None),
                    )
                if extra is not None:
                    extra(ps)
                drain(m, n, ps)

            # ---- a1T[e,s] ----------------------------------------------
            a1s = [a1p.tile([P, N], f32r, name=f"a1_{b}_{m}", tag=f"a1{m}")
                   for m in range(NT)]
            for m in range(NT):
                for n in range(2):
                    mmgroup(ws, f1s, m, n,
                            lambda m_, n_, ps: nc.scalar.copy(
                                a1s[m_][:, n_ * HALF:(n_ + 1) * HALF], ps[:]),
                            "a1")

            # ---- cc[s,t] + vmax stat tile -------------------------------
            ccs = [ccp.tile([P, N], f32r, name=f"cc_{b}_{m}", tag=f"cc{m}")
                   for m in range(NT)]
            nvmax_pp = [smallp.tile([P, 1], f32r, name=f"nvmax_{b}_{m}",
                                    tag=f"vmaxpp{m}") for m in range(NT)]
            scr_v = dscrp.tile([1, N], f32r, name=f"scr_v{b}", tag="scr_v")
            nvrow = statp.tile([1, N], f32r, name=f"nvrow{b}", tag="nvrow")

            def cc_drain(m, n, ps):
                nc.vector.tensor_copy(ccs[m][:, n * HALF:(n + 1) * HALF], ps[:])

            for m in range(NT):
                for n in range(2):
                    mmgroup(a1s, f2s, m, n, cc_drain, "cc")
                nc.vector.tensor_reduce(
                    out=nvmax_pp[m][:], in_=ccs[m][:].bitcast(f32),
                    axis=mybir.AxisListType.X, op=mybir.AluOpType.max, negate=True)
                nc.sync.dma_start(
                    scr_v[0:1, m * P:(m + 1) * P].rearrange("one (p x) -> (one p) x", p=P),
                    nvmax_pp[m][:])
            for h in range(2):
                nc.sync.dma_start(nvrow[0:1, h * HALF:(h + 1) * HALF],
                                  scr_v[0:1, h * HALF:(h + 1) * HALF])

            # ---- ccT[t,s] - vmax[s], exp-drained -> e2T ------------------
            ccts = [cctp.tile([P, N], f32r, name=f"cct_{b}_{m}", tag=f"cct{m}")
                    for m in range(NT)]

            def cct_drain(m, n, ps):
                nc.scalar.activation(ccts[m][:, n * HALF:(n + 1) * HALF], ps[:], Exp)

            for m in range(NT):
                for n in range(2):
                    mmgroup(f2s, a1s, m, n, cct_drain, "cct",
                            extra=lambda ps, n_=n: nc.tensor.matmul(
                                ps[:], ones_k1[:],
                                nvrow[0:1, n_ * HALF:(n_ + 1) * HALF],
                                start=False, stop=True))

            # ---- amax + e1 = exp(cc - amax), per column half -------------
            for h in range(2):
                sl = slice(h * HALF, (h + 1) * HALF)
                amaxt = statp.tile([P, HALF], f32, name=f"amaxt{b}{h}", tag="amaxt")
                nc.vector.tensor_copy(amaxt[:], ccs[0][:, sl].bitcast(f32))
                for m in range(1, NT):
                    nc.vector.tensor_tensor(
                        out=amaxt[:], in0=amaxt[:], in1=ccs[m][:, sl].bitcast(f32),
                        op=mybir.AluOpType.max)
                amaxB = statp.tile([P, HALF], f32, name=f"amaxB{b}{h}", tag="amaxB")
                nc.gpsimd.partition_all_reduce(
                    amaxB[:], amaxt[:], channels=P, reduce_op=bass_isa.ReduceOp.max)
                for m in range(NT):
                    nc.vector.tensor_tensor(
                        out=ccs[m][:, sl], in0=ccs[m][:, sl].bitcast(f32),
                        in1=amaxB[:], op=mybir.AluOpType.subtract)
                    nc.scalar.activation(ccs[m][:, sl], ccs[m][:, sl].bitcast(f32), Exp)

            # ---- column sums via f32r ones-row matmuls + DRAM bounce -----
            rsa = smallp.tile([P, NT], f32, name=f"rsa{b}", tag="rsa")
            rsv = smallp.tile([P, NT], f32, name=f"rsv{b}", tag="rsv")
            scr_s = dscrp.tile([1, 4 * N], f32, name=f"scr_s{b}", tag="scr_s")

            def colsum_mm(tiles, h, col):
                sps = rowpsp.tile([1, HALF], f32, name="sps", tag="rowps")
                for k in range(NT):
                    nc.tensor.matmul(
                        sps[:], ones_col[:], tiles[k][:, h * HALF:(h + 1) * HALF],
                        start=(k == 0), stop=(k == NT - 1))
                # hop through SBUF (DMA cannot read PSUM); reuse a dead a1 slot
                srow = a1p.tile([1, HALF], f32, name="sumrow", tag="a10")
                nc.scalar.copy(srow[:], sps[:])
                nc.sync.dma_start(scr_s[0:1, col * HALF:(col + 1) * HALF], srow[:])

            def ret_drain(out_d, rs):
                def d(m, n, ps):
                    ot = oretp.tile([P, HALF], f32, name="oret", tag="oret")
                    nc.scalar.activation(ot[:], ps[:], Copy,
                                         bias=0.0, scale=rs[:, m:m + 1])
                    nc.sync.dma_start(
                        out_d[b, m * P:(m + 1) * P, n * HALF:(n + 1) * HALF], ot[:])
                return d

            # vsum is ready first (e2T drains straight out of ccT); run the
            # first half of ret2 while the e1 exp/asum path finishes, then
            # ret1 (frees f1 slots for the next batch), then the rest of ret2.
            for h in range(2):
                colsum_mm(ccts, h, 2 + h)     # vsum halves at scr_s cols 2,3
            for h in range(2):
                rd = scr_s[0:1, (2 + h) * HALF:(3 + h) * HALF].rearrange(
                    "one (m p) -> (one p) m", p=P)
                nc.sync.dma_start(rsv[:, 4 * h:4 * h + 4], rd)
            nc.vector.reciprocal(rsv[:], rsv[:])

            for m in range(0, 4):
                for n in range(2):
                    mmgroup(ccts, f2s, m, n, ret_drain(o2_d, rsv), "r2a")

            for h in range(2):
                colsum_mm(ccs, h, h)          # asum halves at scr_s cols 0,1
            for h in range(2):
                rd = scr_s[0:1, h * HALF:(h + 1) * HALF].rearrange(
                    "one (m p) -> (one p) m", p=P)
                nc.sync.dma_start(rsa[:, 4 * h:4 * h + 4], rd)
            nc.vector.reciprocal(rsa[:], rsa[:])

            for m in range(NT):
                for n in range(2):
                    mmgroup(ccs, f1s, m, n, ret_drain(o1_d, rsa), "r1")

            for m in range(4, NT):
                for n in range(2):
                    mmgroup(ccts, f2s, m, n, ret_drain(o2_d, rsv), "r2b")

    nc.compile()
    return nc


_NC = None
TRACE = False
LAST = None


def _get_nc():
    global _NC
    if _NC is None:
        _NC = _build()
    return _NC


def kernel(f1_norm, f2_norm, corr_weights):
    f1_norm = np.ascontiguousarray(f1_norm, dtype=np.float32)
    f2_norm = np.ascontiguousarray(f2_norm, dtype=np.float32)
    w = np.ascontiguousarray(corr_weights, dtype=np.float32)
    B = f1_norm.shape[0]
    assert B == NB * NCORES

    # host-side feature-major transposes: f1t[b] = f1[b].T
    f1t = np.ascontiguousarray(np.swapaxes(f1_norm, 1, 2))
    f2t = np.ascontiguousarray(np.swapaxes(f2_norm, 1, 2))

    nc = _get_nc()
    in_maps = [
        {"f1t": f1t[c * NB:(c + 1) * NB], "f2t": f2t[c * NB:(c + 1) * NB], "w": w}
        for c in range(NCORES)
    ]
    res = run_bass_kernel_spmd(nc, in_maps, core_ids=list(range(NCORES)), trace=TRACE)
    global LAST
    LAST = res
    out1 = np.concatenate([res.results[c]["o1"] for c in range(NCORES)], axis=0)
    out2 = np.concatenate([res.results[c]["o2"] for c in range(NCORES)], axis=0)
    return out1, out2


# revision 25
# speedup vs baseline: 1.2376x; 1.0016x over previous
"""Trainium2 Bass kernel for nn_CAM_41377714929724 (CAM cross-attention module).

  a1  = f1 @ W                      [B,S,D]
  cc  = a1 @ f2^T                   [B,S,S]
  aatt = softmax(cc, axis=s)        (over rows)
  vatt = softmax(cc, axis=t).T      (over cols, transposed)
  out1 = (f1 @ aatt).swap(1,2)      [B,S,S]
  out2 = (f2 @ vatt).swap(1,2)      [B,S,S]

Sharding: pure data parallelism, 2 batches per core on 8 cores; W replicated.

Per core/batch dataflow (all matmuls fp32r = full PE rate, fp32 PSUM accum):
  a1T[e,s] = sum_d W[d,e] f1T[d,s]          (lhsT=W,    rhs=f1T)
  cc [s,t] = sum_e a1T[e,s] f2T[e,t]        (lhsT=a1T,  rhs=f2T)
  vmax[s]  = max_t cc   (DVE free-dim reduce -> [128,8] stat tile
                         -> PE-transpose -> -vmax row, no DRAM bounce)
  ccT[t,s] = sum_e f2T[e,t] a1T[e,s] - vmax[s]
             (K=1 ones x (-vmax row) matmul appended to the accumulation;
              PSUM drain IS the exp -> e2T[t,s] in one ACT op)
  amax[t]  = max_s cc   (DVE max-combine of 8 tiles + 1 gpsimd partition allreduce)
  e1 [u,t] = exp(cc - amax[t])  in place    (DVE sub + ACT exp)
  asum[x]  = sum_u e1[u,x]  (8x N=1 matmul vs ones column -> [128,1] PSUM,
                             per-partition x directly; 1/asum via DVE recip)
  vsum[s]  = sum_u e2T[u,s] (same)
  out1[x,s] = (sum_u e1[u,x] f1T[u,s]) * (1/asum[x])  (scale fused in PSUM drain)
  out2[s,t] = (sum_u e2T[u,s] f2T[u,t]) * (1/vsum[s])

Column-halved stats keep the PE dense: ret matmuls of one half start while the
other half's stats are in flight.
"""

import numpy as np
from contextlib import ExitStack

import concourse.bass as bass
import concourse.tile as tile
from concourse import bacc, mybir, bass_isa
from concourse.bass_utils import run_bass_kernel_spmd

f32 = mybir.dt.float32
f32r = mybir.dt.float32r

P = 128
N = 1024
NT = N // P          # 8 tiles per matrix dim
NB = 2               # batches per core
NCORES = 8
HALF = 512           # matmul moving free dim / psum bank
Exp = mybir.ActivationFunctionType.Exp
Copy = mybir.ActivationFunctionType.Copy


def _build():
    nc = bacc.Bacc("TRN2", target_bir_lowering=False, debug=False, num_devices=NCORES)

    f1t_d = nc.dram_tensor("f1t", [NB, N, N], f32r, kind="ExternalInput").ap()
    f2t_d = nc.dram_tensor("f2t", [NB, N, N], f32r, kind="ExternalInput").ap()
    w_d = nc.dram_tensor("w", [N, N], f32r, kind="ExternalInput").ap()
    o1_d = nc.dram_tensor("o1", [NB, N, N], f32, kind="ExternalOutput").ap()
    o2_d = nc.dram_tensor("o2", [NB, N, N], f32, kind="ExternalOutput").ap()

    with tile.TileContext(nc) as tc, ExitStack() as ctx:
        wp = ctx.enter_context(tc.tile_pool(name="wp", bufs=1))
        f1p = ctx.enter_context(tc.tile_pool(name="f1p", bufs=1))
        f2p = ctx.enter_context(tc.tile_pool(name="f2p", bufs=1))
        a1p = ctx.enter_context(tc.tile_pool(name="a1p", bufs=1))
        ccp = ctx.enter_context(tc.tile_pool(name="ccp", bufs=1))
        cctp = ctx.enter_context(tc.tile_pool(name="cctp", bufs=1))
        statp = ctx.enter_context(tc.tile_pool(name="statp", bufs=1))
        smallp = ctx.enter_context(tc.tile_pool(name="smallp", bufs=1))
        oretp = ctx.enter_context(tc.tile_pool(name="oretp", bufs=2))
        psp = ctx.enter_context(tc.tile_pool(name="psp", bufs=8, space="PSUM"))
        dscrp = ctx.enter_context(tc.tile_pool(name="dscrp", bufs=2, space="DRAM"))

        # constants: fp32r ones (memset can't write f32r), fp32 identity
        ones_f32r_ = smallp.tile([1, P], f32, name="ones_f32r_", tag="ones_f32r_")
        nc.vector.memset(ones_f32r_[:], 1.0)
        ones_k1 = smallp.tile([1, P], f32r, name="ones_k1", tag="ones_k1")
        nc.scalar.copy(ones_k1[:], ones_f32r_[:])
        ones_f32c_ = smallp.tile([P, 1], f32, name="ones_f32c_", tag="ones_f32c_")
        nc.vector.memset(ones_f32c_[:], 1.0)
        ones_col = smallp.tile([P, 1], f32r, name="ones_col", tag="ones_col")
        nc.scalar.copy(ones_col[:], ones_f32c_[:])

        # W is shared by both batches: load once
        ws = []
        f1s_by_b = {}
        for k in range(NT):
            wk = wp.tile([P, N], f32r, name=f"w{k}", tag=f"w{k}")
            nc.sync.dma_start(wk[:], w_d[k * P:(k + 1) * P, :])
            ws.append(wk)
            f1k = f1p.tile([P, N], f32r, name=f"f1_0_{k}", tag=f"f1{k}")
            nc.sync.dma_start(f1k[:], f1t_d[0, k * P:(k + 1) * P, :])
            f1s_by_b.setdefault(0, []).append(f1k)

        for b in range(NB):
            # ---- loads -------------------------------------------------
            if b == 0:
                f1s = f1s_by_b[0]
            else:
                f1s = []
                for k in range(NT):
                    f1k = f1p.tile([P, N], f32r, name=f"f1_{b}_{k}", tag=f"f1{k}")
                    nc.sync.dma_start(f1k[:], f1t_d[b, k * P:(k + 1) * P, :])
                    f1s.append(f1k)
            f2s = []
            for k in range(NT):
                f2k = f2p.tile([P, N], f32r, name=f"f2_{b}_{k}", tag=f"f2{k}")
                nc.sync.dma_start(f2k[:], f2t_d[b, k * P:(k + 1) * P, :])
                f2s.append(f2k)

            def mmgroup(lhs_tiles, rhs_tiles, m, n, drain, tagpfx, extra=None):
                ps = psp.tile([P, HALF], f32, name=f"ps_{tagpfx}", tag="ps")
                for k in range(NT):
                    nc.tensor.matmul(
                        ps[:],
                        lhs_tiles[k][:, m * P:(m + 1) * P],
                        rhs_tiles[k][:, n * HALF:(n + 1) * HALF],
                        start=(k == 0),
                        stop=(k == NT - 1 and extra is None),
                    )
                if extra is not None:
                    extra(ps)
                drain(m, n, ps)

            # ---- a1T[e,s] ----------------------------------------------
            a1s = [a1p.tile([P, N], f32r, name=f"a1_{b}_{m}", tag=f"a1{m}")
                   for m in range(NT)]
            for m in range(NT):
                for n in range(2):
                    mmgroup(ws, f1s, m, n,
                            lambda m_, n_, ps: nc.scalar.copy(
                                a1s[m_][:, n_ * HALF:(n_ + 1) * HALF], ps[:]),
                            "a1")

            # ---- cc[s,t] + vmax stat tile -------------------------------
            ccs = [ccp.tile([P, N], f32r, name=f"cc_{b}_{m}", tag=f"cc{m}")
                   for m in range(NT)]
            nvmax_pp = [smallp.tile([P, 1], f32r, name=f"nvmax_{b}_{m}",
                                    tag=f"vmaxpp{m}") for m in range(NT)]
            scr_v = dscrp.tile([1, N], f32r, name=f"scr_v{b}", tag="scr_v")
            nvrow = statp.tile([1, N], f32r, name=f"nvrow{b}", tag="nvrow")

            def cc_drain(m, n, ps):
                nc.vector.tensor_copy(ccs[m][:, n * HALF:(n + 1) * HALF], ps[:])

            for m in range(NT):
                for n in range(2):
                    mmgroup(a1s, f2s, m, n, cc_drain, "cc")
                nc.vector.tensor_reduce(
                    out=nvmax_pp[m][:], in_=ccs[m][:].bitcast(f32),
                    axis=mybir.AxisListType.X, op=mybir.AluOpType.max, negate=True)
                nc.sync.dma_start(
                    scr_v[0:1, m * P:(m + 1) * P].rearrange("one (p x) -> (one p) x", p=P),
                    nvmax_pp[m][:])
            for h in range(2):
                nc.sync.dma_start(nvrow[0:1, h * HALF:(h + 1) * HALF],
                                  scr_v[0:1, h * HALF:(h + 1) * HALF])

            # ---- ccT[t,s] - vmax[s], exp-drained -> e2T ------------------
            ccts = [cctp.tile([P, N], f32r, name=f"cct_{b}_{m}", tag=f"cct{m}")
                    for m in range(NT)]

            def cct_drain(m, n, ps):
                nc.scalar.activation(ccts[m][:, n * HALF:(n + 1) * HALF], ps[:], Exp)

            for m in range(NT):
                for n in range(2):
                    mmgroup(f2s, a1s, m, n, cct_drain, "cct",
                            extra=lambda ps, n_=n: nc.tensor.matmul(
                                ps[:], ones_k1[:],
                                nvrow[0:1, n_ * HALF:(n_ + 1) * HALF],
                                start=False, stop=True))

            # ---- amax + e1 = exp(cc - amax), per column half -------------
            for h in range(2):
                sl = slice(h * HALF, (h + 1) * HALF)
                amaxt = statp.tile([P, HALF], f32, name=f"amaxt{b}{h}", tag="amaxt")
                nc.vector.tensor_copy(amaxt[:], ccs[0][:, sl].bitcast(f32))
                for m in range(1, NT):
                    nc.vector.tensor_tensor(
                        out=amaxt[:], in0=amaxt[:], in1=ccs[m][:, sl].bitcast(f32),
                        op=mybir.AluOpType.max)
                amaxB = statp.tile([P, HALF], f32, name=f"amaxB{b}{h}", tag="amaxB")
                nc.gpsimd.partition_all_reduce(
                    amaxB[:], amaxt[:], channels=P, reduce_op=bass_isa.ReduceOp.max)
                for m in range(NT):
                    nc.vector.tensor_tensor(
                        out=ccs[m][:, sl], in0=ccs[m][:, sl].bitcast(f32),
                        in1=amaxB[:], op=mybir.AluOpType.subtract)
                    nc.scalar.activation(ccs[m][:, sl], ccs[m][:, sl].bitcast(f32), Exp)

            # ---- column sums via f32r ones-row matmuls + DRAM bounce -----
            rsa = smallp.tile([P, NT], f32, name=f"rsa{b}", tag="rsa")
            rsv = smallp.tile([P, NT], f32, name=f"rsv{b}", tag="rsv")
            scr_s = dscrp.tile([1, 4 * N], f32, name=f"scr_s{b}", tag="scr_s")

            def colsum_mm(tiles, h, col):
                sps = psp.tile([1, HALF], f32, name="sps", tag="ps")
                for k in range(NT):
                    nc.tensor.matmul(
                        sps[:], ones_col[:], tiles[k][:, h * HALF:(h + 1) * HALF],
                        start=(k == 0), stop=(k == NT - 1))
                # hop through SBUF (DMA cannot read PSUM); reuse a dead a1 slot
                srow = a1p.tile([1, HALF], f32, name="sumrow", tag="a10")
                nc.vector.tensor_copy(srow[:], sps[:])
                nc.sync.dma_start(scr_s[0:1, col * HALF:(col + 1) * HALF], srow[:])

            def ret_drain(out_d, rs, dve=False):
                def d(m, n, ps):
                    ot = oretp.tile([P, HALF], f32, name="oret", tag="oret")
                    if dve:
                        nc.vector.tensor_scalar_mul(ot[:], ps[:], rs[:, m:m + 1])
                    else:
                        nc.scalar.activation(ot[:], ps[:], Copy,
                                             bias=0.0, scale=rs[:, m:m + 1])
                    nc.sync.dma_start(
                        out_d[b, m * P:(m + 1) * P, n * HALF:(n + 1) * HALF], ot[:])
                return d

            # vsum is ready first (e2T drains straight out of ccT); run the
            # first half of ret2 while the e1 exp/asum path finishes, then
            # ret1 (frees f1 slots for the next batch), then the rest of ret2.
            for h in range(2):
                colsum_mm(ccts, h, 2 + h)     # vsum halves at scr_s cols 2,3
            for h in range(2):
                rd = scr_s[0:1, (2 + h) * HALF:(3 + h) * HALF].rearrange(
                    "one (m p) -> (one p) m", p=P)
                nc.sync.dma_start(rsv[:, 4 * h:4 * h + 4], rd)
            nc.vector.reciprocal(rsv[:], rsv[:])

            for m in range(0, 4):
                for n in range(2):
                    mmgroup(ccts, f2s, m, n, ret_drain(o2_d, rsv), "r2a")

            for h in range(2):
                colsum_mm(ccs, h, h)          # asum halves at scr_s cols 0,1
            for h in range(2):
                rd = scr_s[0:1, h * HALF:(h + 1) * HALF].rearrange(
                    "one (m p) -> (one p) m", p=P)
                nc.sync.dma_start(rsa[:, 4 * h:4 * h + 4], rd)
            nc.vector.reciprocal(rsa[:], rsa[:])

            for m in range(NT):
                for n in range(2):
                    mmgroup(ccs, f1s, m, n, ret_drain(o1_d, rsa, dve=True), "r1")

            for m in range(4, NT):
                for n in range(2):
                    mmgroup(ccts, f2s, m, n, ret_drain(o2_d, rsv), "r2b")

    nc.compile()
    return nc


_NC = None
TRACE = False
LAST = None


def _get_nc():
    global _NC
    if _NC is None:
        _NC = _build()
    return _NC


def kernel(f1_norm, f2_norm, corr_weights):
    f1_norm = np.ascontiguousarray(f1_norm, dtype=np.float32)
    f2_norm = np.ascontiguousarray(f2_norm, dtype=np.float32)
    w = np.ascontiguousarray(corr_weights, dtype=np.float32)
    B = f1_norm.shape[0]
    assert B == NB * NCORES

    # host-side feature-major transposes: f1t[b] = f1[b].T
    f1t = np.ascontiguousarray(np.swapaxes(f1_norm, 1, 2))
    f2t = np.ascontiguousarray(np.swapaxes(f2_norm, 1, 2))

    nc = _get_nc()
    in_maps = [
        {"f1t": f1t[c * NB:(c + 1) * NB], "f2t": f2t[c * NB:(c + 1) * NB], "w": w}
        for c in range(NCORES)
    ]
    res = run_bass_kernel_spmd(nc, in_maps, core_ids=list(range(NCORES)), trace=TRACE)
    global LAST
    LAST = res
    out1 = np.concatenate([res.results[c]["o1"] for c in range(NCORES)], axis=0)
    out2 = np.concatenate([res.results[c]["o2"] for c in range(NCORES)], axis=0)
    return out1, out2


# revision 30
# speedup vs baseline: 1.3785x; 1.1138x over previous
"""Trainium2 Bass kernel for nn_CAM_41377714929724 (CAM cross-attention module).

  a1  = f1 @ W                      [B,S,D]
  cc  = a1 @ f2^T                   [B,S,S]
  aatt = softmax(cc, axis=s)        (over rows)
  vatt = softmax(cc, axis=t).T      (over cols, transposed)
  out1 = (f1 @ aatt).swap(1,2)      [B,S,S]
  out2 = (f2 @ vatt).swap(1,2)      [B,S,S]

Sharding: pure data parallelism, 2 batches per core on 8 cores; W replicated.

Per core/batch dataflow (all matmuls fp32r = full PE rate, fp32 PSUM accum):
  a1T[e,s] = sum_d W[d,e] f1T[d,s]          (lhsT=W,    rhs=f1T)
  cc [s,t] = sum_e a1T[e,s] f2T[e,t]        (lhsT=a1T,  rhs=f2T)
  vmax[s]  = max_t cc   (DVE free-dim reduce -> [128,8] stat tile
                         -> PE-transpose -> -vmax row, no DRAM bounce)
  ccT[t,s] = sum_e f2T[e,t] a1T[e,s] - vmax[s]
             (K=1 ones x (-vmax row) matmul appended to the accumulation;
              PSUM drain IS the exp -> e2T[t,s] in one ACT op)
  amax[t]  = max_s cc   (DVE max-combine of 8 tiles + 1 gpsimd partition allreduce)
  e1 [u,t] = exp(cc - amax[t])  in place    (DVE sub + ACT exp)
  asum[x]  = sum_u e1[u,x]  (8x N=1 matmul vs ones column -> [128,1] PSUM,
                             per-partition x directly; 1/asum via DVE recip)
  vsum[s]  = sum_u e2T[u,s] (same)
  out1[x,s] = (sum_u e1[u,x] f1T[u,s]) * (1/asum[x])  (scale fused in PSUM drain)
  out2[s,t] = (sum_u e2T[u,s] f2T[u,t]) * (1/vsum[s])

Column-halved stats keep the PE dense: ret matmuls of one half start while the
other half's stats are in flight.
"""

import numpy as np
from contextlib import ExitStack

import concourse.bass as bass
import concourse.tile as tile
from concourse import bacc, mybir, bass_isa
from concourse.bass_utils import run_bass_kernel_spmd

f32 = mybir.dt.float32
f32r = mybir.dt.float32r

P = 128
N = 1024
NT = N // P          # 8 tiles per matrix dim
NB = 2               # batches per core
NCORES = 8
HALF = 512           # matmul moving free dim / psum bank
Exp = mybir.ActivationFunctionType.Exp
Copy = mybir.ActivationFunctionType.Copy


def _build():
    nc = bacc.Bacc("TRN2", target_bir_lowering=False, debug=False, num_devices=NCORES)

    f1t_d = nc.dram_tensor("f1t", [NB, N, N], f32r, kind="ExternalInput").ap()
    f2t_d = nc.dram_tensor("f2t", [NB, N, N], f32r, kind="ExternalInput").ap()
    w_d = nc.dram_tensor("w", [N, N], f32r, kind="ExternalInput").ap()
    id_d = nc.dram_tensor("ident", [P, P], f32r, kind="ExternalInput").ap()
    o1_d = nc.dram_tensor("o1", [NB, N, N], f32, kind="ExternalOutput").ap()
    o2_d = nc.dram_tensor("o2", [NB, N, N], f32, kind="ExternalOutput").ap()

    with tile.TileContext(nc) as tc, ExitStack() as ctx:
        wp = ctx.enter_context(tc.tile_pool(name="wp", bufs=1))
        f1p = ctx.enter_context(tc.tile_pool(name="f1p", bufs=1))
        f2p = ctx.enter_context(tc.tile_pool(name="f2p", bufs=1))
        a1p = ctx.enter_context(tc.tile_pool(name="a1p", bufs=1))
        ccp = ctx.enter_context(tc.tile_pool(name="ccp", bufs=1))
        cctp = ctx.enter_context(tc.tile_pool(name="cctp", bufs=1))
        statp = ctx.enter_context(tc.tile_pool(name="statp", bufs=1))
        smallp = ctx.enter_context(tc.tile_pool(name="smallp", bufs=1))
        oretp = ctx.enter_context(tc.tile_pool(name="oretp", bufs=2))
        psp = ctx.enter_context(tc.tile_pool(name="psp", bufs=8, space="PSUM"))
        dscrp = ctx.enter_context(tc.tile_pool(name="dscrp", bufs=2, space="DRAM"))

        # constants: fp32r ones (memset can't write f32r), fp32 identity
        ones_f32r_ = smallp.tile([1, P], f32, name="ones_f32r_", tag="ones_f32r_")
        nc.vector.memset(ones_f32r_[:], 1.0)
        ones_k1 = smallp.tile([1, P], f32r, name="ones_k1", tag="ones_k1")
        nc.scalar.copy(ones_k1[:], ones_f32r_[:])
        ones_f32c_ = smallp.tile([P, 1], f32, name="ones_f32c_", tag="ones_f32c_")
        nc.vector.memset(ones_f32c_[:], 1.0)
        ones_col = smallp.tile([P, 1], f32r, name="ones_col", tag="ones_col")
        nc.scalar.copy(ones_col[:], ones_f32c_[:])
        ident = smallp.tile([P, P], f32r, name="ident", tag="ident")
        nc.sync.dma_start(ident[:], id_d[:, :])

        # W is shared by both batches: load once
        ws = []
        f1s_by_b = {}
        for k in range(NT):
            wk = wp.tile([P, N], f32r, name=f"w{k}", tag=f"w{k}")
            nc.sync.dma_start(wk[:], w_d[k * P:(k + 1) * P, :])
            ws.append(wk)
            f1k = f1p.tile([P, N], f32r, name=f"f1_0_{k}", tag=f"f1{k}")
            nc.sync.dma_start(f1k[:], f1t_d[0, k * P:(k + 1) * P, :])
            f1s_by_b.setdefault(0, []).append(f1k)

        for b in range(NB):
            # ---- loads -------------------------------------------------
            if b == 0:
                f1s = f1s_by_b[0]
            else:
                f1s = []
                for k in range(NT):
                    f1k = f1p.tile([P, N], f32r, name=f"f1_{b}_{k}", tag=f"f1{k}")
                    nc.sync.dma_start(f1k[:], f1t_d[b, k * P:(k + 1) * P, :])
                    f1s.append(f1k)
            f2s = []
            for k in range(NT):
                f2k = f2p.tile([P, N], f32r, name=f"f2_{b}_{k}", tag=f"f2{k}")
                nc.sync.dma_start(f2k[:], f2t_d[b, k * P:(k + 1) * P, :])
                f2s.append(f2k)

            def mmgroup(lhs_tiles, rhs_tiles, m, n, drain, tagpfx, extra=None):
                ps = psp.tile([P, HALF], f32, name=f"ps_{tagpfx}", tag="ps")
                for k in range(NT):
                    nc.tensor.matmul(
                        ps[:],
                        lhs_tiles[k][:, m * P:(m + 1) * P],
                        rhs_tiles[k][:, n * HALF:(n + 1) * HALF],
                        start=(k == 0),
                        stop=(k == NT - 1 and extra is None),
                    )
                if extra is not None:
                    extra(ps)
                drain(m, n, ps)

            # ---- a1T[e,s] ----------------------------------------------
            a1s = [a1p.tile([P, N], f32r, name=f"a1_{b}_{m}", tag=f"a1{m}")
                   for m in range(NT)]
            for m in range(NT):
                for n in range(2):
                    mmgroup(ws, f1s, m, n,
                            lambda m_, n_, ps: nc.scalar.copy(
                                a1s[m_][:, n_ * HALF:(n_ + 1) * HALF], ps[:]),
                            "a1")

            # ---- cc[s,t] + vmax stat tile -------------------------------
            ccs = [ccp.tile([P, N], f32r, name=f"cc_{b}_{m}", tag=f"cc{m}")
                   for m in range(NT)]
            nvmax_pp = [smallp.tile([P, 1], f32r, name=f"nvmax_{b}_{m}",
                                    tag=f"vmaxpp{m}") for m in range(NT)]
            scr_v = dscrp.tile([1, N], f32r, name=f"scr_v{b}", tag="scr_v")
            nvrow = statp.tile([1, N], f32r, name=f"nvrow{b}", tag="nvrow")

            def cc_drain(m, n, ps):
                nc.vector.tensor_copy(ccs[m][:, n * HALF:(n + 1) * HALF], ps[:])

            for m in range(NT):
                for n in range(2):
                    mmgroup(a1s, f2s, m, n, cc_drain, "cc")
                nc.vector.tensor_reduce(
                    out=nvmax_pp[m][:], in_=ccs[m][:].bitcast(f32),
                    axis=mybir.AxisListType.X, op=mybir.AluOpType.max, negate=True)
                nc.sync.dma_start(
                    scr_v[0:1, m * P:(m + 1) * P].rearrange("one (p x) -> (one p) x", p=P),
                    nvmax_pp[m][:])
            for h in range(2):
                nc.sync.dma_start(nvrow[0:1, h * HALF:(h + 1) * HALF],
                                  scr_v[0:1, h * HALF:(h + 1) * HALF])

            # ---- ccT[t,s] - vmax[s] via PE transpose of cc, exp-drained --
            # ccT[m-tile][:, q-block of half n] = (cc[4n+q][:, m*128..])^T
            ccts = [cctp.tile([P, N], f32r, name=f"cct_{b}_{m}", tag=f"cct{m}")
                    for m in range(NT)]
            for m in range(NT):
                for n in range(2):
                    ps = psp.tile([P, HALF], f32r, name="ps_t", tag="ps")
                    for q in range(4):
                        nc.tensor.matmul(
                            ps[:, q * P:(q + 1) * P],
                            ccs[4 * n + q][:, m * P:(m + 1) * P], ident[:],
                            is_transpose=True, start=(q == 0), stop=False)
                    nc.tensor.matmul(
                        ps[:].bitcast(f32), ones_k1[:],
                        nvrow[0:1, n * HALF:(n + 1) * HALF],
                        start=False, stop=True)
                    nc.scalar.activation(ccts[m][:, n * HALF:(n + 1) * HALF],
                                         ps[:].bitcast(f32), Exp)

            # ---- amax + e1 = exp(cc - amax), per column half -------------
            for h in range(2):
                sl = slice(h * HALF, (h + 1) * HALF)
                amaxt = statp.tile([P, HALF], f32, name=f"amaxt{b}{h}", tag="amaxt")
                nc.vector.tensor_copy(amaxt[:], ccs[0][:, sl].bitcast(f32))
                for m in range(1, NT):
                    nc.vector.tensor_tensor(
                        out=amaxt[:], in0=amaxt[:], in1=ccs[m][:, sl].bitcast(f32),
                        op=mybir.AluOpType.max)
                amaxB = statp.tile([P, HALF], f32, name=f"amaxB{b}{h}", tag="amaxB")
                nc.gpsimd.partition_all_reduce(
                    amaxB[:], amaxt[:], channels=P, reduce_op=bass_isa.ReduceOp.max)
                for m in range(NT):
                    nc.vector.tensor_tensor(
                        out=ccs[m][:, sl], in0=ccs[m][:, sl].bitcast(f32),
                        in1=amaxB[:], op=mybir.AluOpType.subtract)
                    nc.scalar.activation(ccs[m][:, sl], ccs[m][:, sl].bitcast(f32), Exp)

            # ---- column sums via f32r ones-row matmuls + DRAM bounce -----
            rsa = smallp.tile([P, NT], f32, name=f"rsa{b}", tag="rsa")
            rsv = smallp.tile([P, NT], f32, name=f"rsv{b}", tag="rsv")
            scr_s = dscrp.tile([1, 4 * N], f32, name=f"scr_s{b}", tag="scr_s")

            def colsum_mm(tiles, h, col):
                sps = psp.tile([1, HALF], f32, name="sps", tag="ps")
                for k in range(NT):
                    nc.tensor.matmul(
                        sps[:], ones_col[:], tiles[k][:, h * HALF:(h + 1) * HALF],
                        start=(k == 0), stop=(k == NT - 1))
                # hop through SBUF (DMA cannot read PSUM); reuse a dead a1 slot
                srow = a1p.tile([1, HALF], f32, name="sumrow", tag="a10")
                nc.vector.tensor_copy(srow[:], sps[:])
                nc.sync.dma_start(scr_s[0:1, col * HALF:(col + 1) * HALF], srow[:])

            def ret_drain(out_d, rs, dve=False):
                def d(m, n, ps):
                    ot = oretp.tile([P, HALF], f32, name="oret", tag="oret")
                    if dve:
                        nc.vector.tensor_scalar_mul(ot[:], ps[:], rs[:, m:m + 1])
                    else:
                        nc.scalar.activation(ot[:], ps[:], Copy,
                                             bias=0.0, scale=rs[:, m:m + 1])
                    nc.sync.dma_start(
                        out_d[b, m * P:(m + 1) * P, n * HALF:(n + 1) * HALF], ot[:])
                return d

            # vsum is ready first (e2T drains straight out of ccT); run the
            # first half of ret2 while the e1 exp/asum path finishes, then
            # ret1 (frees f1 slots for the next batch), then the rest of ret2.
            for h in range(2):
                colsum_mm(ccts, h, 2 + h)     # vsum halves at scr_s cols 2,3
            for h in range(2):
                rd = scr_s[0:1, (2 + h) * HALF:(3 + h) * HALF].rearrange(
                    "one (m p) -> (one p) m", p=P)
                nc.sync.dma_start(rsv[:, 4 * h:4 * h + 4], rd)
            nc.vector.reciprocal(rsv[:], rsv[:])

            for m in range(0, 4):
                for n in range(2):
                    mmgroup(ccts, f2s, m, n, ret_drain(o2_d, rsv, dve=True), "r2a")

            for h in range(2):
                colsum_mm(ccs, h, h)          # asum halves at scr_s cols 0,1
            for h in range(2):
                rd = scr_s[0:1, h * HALF:(h + 1) * HALF].rearrange(
                    "one (m p) -> (one p) m", p=P)
                nc.sync.dma_start(rsa[:, 4 * h:4 * h + 4], rd)
            nc.vector.reciprocal(rsa[:], rsa[:])

            for m in range(NT):
                for n in range(2):
                    mmgroup(ccs, f1s, m, n, ret_drain(o1_d, rsa), "r1")

            for m in range(4, NT):
                for n in range(2):
                    mmgroup(ccts, f2s, m, n, ret_drain(o2_d, rsv, dve=True), "r2b")

    nc.compile()
    return nc


_NC = None
TRACE = False
LAST = None


def _get_nc():
    global _NC
    if _NC is None:
        _NC = _build()
    return _NC


def kernel(f1_norm, f2_norm, corr_weights):
    f1_norm = np.ascontiguousarray(f1_norm, dtype=np.float32)
    f2_norm = np.ascontiguousarray(f2_norm, dtype=np.float32)
    w = np.ascontiguousarray(corr_weights, dtype=np.float32)
    B = f1_norm.shape[0]
    assert B == NB * NCORES

    # host-side feature-major transposes: f1t[b] = f1[b].T
    f1t = np.ascontiguousarray(np.swapaxes(f1_norm, 1, 2))
    f2t = np.ascontiguousarray(np.swapaxes(f2_norm, 1, 2))
    ident = np.eye(P, dtype=np.float32)

    nc = _get_nc()
    in_maps = [
        {"f1t": f1t[c * NB:(c + 1) * NB], "f2t": f2t[c * NB:(c + 1) * NB],
         "w": w, "ident": ident}
        for c in range(NCORES)
    ]
    res = run_bass_kernel_spmd(nc, in_maps, core_ids=list(range(NCORES)), trace=TRACE)
    global LAST
    LAST = res
    out1 = np.concatenate([res.results[c]["o1"] for c in range(NCORES)], axis=0)
    out2 = np.concatenate([res.results[c]["o2"] for c in range(NCORES)], axis=0)
    return out1, out2
